# revision 5
# baseline (speedup 1.0000x reference)
"""Kendall distance kernel for Trainium2, SPMD over 8 NeuronCores.

Math: for X (B=64, T=256, N=64),
  C[i,j] = sum_{a,b,t} sign(X[b,t,i]-X[a,t,i]) * sign(X[b,t,j]-X[a,t,j])
  D = (1 - C/2016) * (1 - eye(N));  output = broadcast D to (B, N, N).

Device work: Gram of the sign tensor over all unordered batch pairs
(2016 pairs x 256 t rows), sharded across 8 cores by cyclic
batch-difference class: core c handles ring-offset classes
d in {4c+1 .. 4c+4}; class 32 (core 7, slot 3) covers its pairs twice,
so j=3 chunks go to a second PSUM accumulator the host halves.

Key trick vs the sub+sign baseline: inputs are per-column RANKS, which
are distinct integers, so sign(r_b - r_a) = 2*I(r_b > r_a) - 1 exactly.
The device computes ONE DVE `is_gt` op per chunk (u in {0,1} bf16) and
Grams u directly on PE.  The host expands
  sum s_i s_j = 4*sum u_i u_j - 2*(S_i + S_j) + count
with S[i] = row-sums of u computed on host from the ranks.  This halves
DVE work and removes all ACT usage (no Sign table load).

Slots are graduated (4,12,16,16,16 batch-blocks): the first chunks need
only a small DMA so the DVE pipeline starts ~5us earlier.  Slot-0
pieces ride the vector queue (DVE is idle until they land anyway);
later slots spread over scalar/gpsimd/tensor/sync queues.
"""

import numpy as np
import ml_dtypes

import concourse.bass as bass  # noqa: F401
import concourse.bacc as bacc
import concourse.tile as tile
from concourse import mybir
from concourse.bass_utils import run_bass_kernel_spmd

B, T, N = 64, 256, 64
P = 128
TH = T // P                   # 2
NCORES = 8
NSLOT = 4                     # classes per core (j = 0..3)
SLOT_NB = (4, 12, 16, 16, 16)  # batch-blocks per slot (sum = 64)
NSLOTS = len(SLOT_NB)
SLOT_START = tuple(int(x) for x in np.cumsum((0,) + SLOT_NB[:-1]))
WEXT = NSLOT - 1              # extra blocks per x2 window (3)
BFD = TH * N                  # free elems per block (128)
PAIRS_HALF = 1008.0

_CACHE = {}


def _build_nc():
    nc = bacc.Bacc(
        "TRN2",
        target_bir_lowering=False,
        debug=False,
        num_devices=NCORES,
    )
    f32 = mybir.dt.float32
    bf16 = mybir.dt.bfloat16
    x1_dram = [
        nc.dram_tensor(f"x1s{s}", [P, nb * BFD], bf16, kind="ExternalInput")
        for s, nb in enumerate(SLOT_NB)
    ]
    x2_dram = [
        nc.dram_tensor(f"x2s{s}", [P, (nb + WEXT) * BFD], bf16, kind="ExternalInput")
        for s, nb in enumerate(SLOT_NB)
    ]
    out_dram = nc.dram_tensor("out", [P, 2 * P], f32, kind="ExternalOutput")

    with tile.TileContext(nc) as tc:
        with (
            tc.tile_pool(name="xpool", bufs=1) as xpool,
            tc.tile_pool(name="spool", bufs=6) as spool,
            tc.tile_pool(name="psum", bufs=2, space="PSUM") as psum,
            tc.tile_pool(name="opool", bufs=1) as opool,
        ):
            x1t = [
                xpool.tile([P, nb * BFD], bf16, tag=f"x1t{s}", name=f"x1t{s}")
                for s, nb in enumerate(SLOT_NB)
            ]
            x2t = [
                xpool.tile([P, (nb + WEXT) * BFD], bf16, tag=f"x2t{s}", name=f"x2t{s}")
                for s, nb in enumerate(SLOT_NB)
            ]
            # Queue plan (only scalar/gpsimd/sync can issue DMAs): slot 0's
            # small pieces lead the scalar queue so DVE starts early; x2
            # pieces (bigger) ride gpsimd; slot 4 (consumed last) on sync.
            nc.scalar.dma_start(x2t[0][:, :], x2_dram[0][:, :])
            nc.scalar.dma_start(x1t[0][:, :], x1_dram[0][:, :])
            for s in (1, 2, 3):
                nc.gpsimd.dma_start(x2t[s][:, :], x2_dram[s][:, :])
                nc.scalar.dma_start(x1t[s][:, :], x1_dram[s][:, :])
            nc.sync.dma_start(x1t[4][:, :], x1_dram[4][:, :])
            nc.sync.dma_start(x2t[4][:, :], x2_dram[4][:, :])

            c1_ps = psum.tile([P, P], f32, tag="c1")
            c2_ps = psum.tile([P, P], f32, tag="c2")
            n1 = (NSLOT - 1) * B // 2 * TH   # matmuls into c1 (192)
            n2 = B // 2 * TH                 # matmuls into c2 (64)
            k1 = k2 = 0
            for s, nb in enumerate(SLOT_NB):
                cfd = nb * BFD
                pmt = nb * TH // 2
                for j in range(NSLOT):
                    idx = s * NSLOT + j
                    sign = spool.tile([P, cfd], bf16, tag=f"sg{s}", name=f"sign{idx}")
                    nc.vector.tensor_tensor(
                        sign[:, :],
                        x2t[s][:, j * BFD:j * BFD + cfd],
                        x1t[s][:, :],
                        op=mybir.AluOpType.is_gt,
                    )
                    for m in range(pmt):
                        w_tile = sign[:, m * P:(m + 1) * P]
                        if j < NSLOT - 1:
                            st, sp = k1 == 0, k1 == n1 - 1
                            k1 += 1
                            acc = c1_ps
                        else:
                            st, sp = k2 == 0, k2 == n2 - 1
                            k2 += 1
                            acc = c2_ps
                        nc.tensor.matmul(
                            acc[:, :], w_tile, w_tile, start=st, stop=sp
                        )

            out_sb = opool.tile([P, 2 * P], f32)
            nc.vector.tensor_copy(out_sb[:, 0:P], c1_ps[:, :])
            nc.vector.tensor_copy(out_sb[:, P:2 * P], c2_ps[:, :])
            nc.sync.dma_start(out_dram[:, :], out_sb[:, :])

    nc.compile()
    return nc


def _get_nc():
    if "nc" not in _CACHE:
        _CACHE["nc"] = _build_nc()
    return _CACHE["nc"]


def _ranks(X):
    """Per-(t,i)-column batch ranks, 0..B-1, exact in bf16."""
    order = np.argsort(X, axis=0, kind="stable")
    ranks = np.empty_like(order)
    np.put_along_axis(
        ranks, order, np.arange(B, dtype=order.dtype)[:, None, None], axis=0
    )
    return ranks.astype(np.float32)


def _to_sbuf_layout(blocks):
    nb = blocks.shape[0]
    return np.ascontiguousarray(
        blocks.reshape(nb, TH, P, N)
        .transpose(2, 0, 1, 3)
        .reshape(P, nb * BFD)
        .astype(ml_dtypes.bfloat16)
    )


def _prep_core_inputs(R, c):
    r = np.roll(R, -(NSLOT * c + 1), axis=0)
    ext = np.concatenate([r, r[:WEXT]], axis=0)  # 67 blocks
    ins = {}
    for s, nb in enumerate(SLOT_NB):
        st = SLOT_START[s]
        ins[f"x1s{s}"] = _to_sbuf_layout(R[st:st + nb])
        ins[f"x2s{s}"] = _to_sbuf_layout(ext[st:st + nb + WEXT])
    return ins


def _u_rowsums(R):
    """Weighted row-sums S[i] = sum over device-covered ordered ring pairs
    (a, a+d), d=1..32 (d=32 halved), of I(R[a+d,t,i] > R[a,t,i])."""
    S = np.zeros(N, dtype=np.float64)
    for d in range(1, 33):
        w = 0.5 if d == 32 else 1.0
        S += w * (np.roll(R, -d, axis=0) > R).sum(axis=(0, 1))
    return S


def _tie_correction(X, ranks):
    """Exact fix for within-column value ties: the rank-sign kernel counts
    sign(rank diff)=+-1 where the true sign is 0."""
    C_fix = np.zeros((N, N), dtype=np.float64)
    Xs = np.sort(X, axis=0)
    t_idx, i_idx = np.nonzero((Xs[1:] == Xs[:-1]).any(axis=0))
    events = {}
    for t, i in zip(t_idx, i_idx):
        col = X[:, t, i]
        order = np.argsort(col, kind="stable")
        sc = col[order]
        for k in np.nonzero(sc[1:] == sc[:-1])[0]:
            a, b = order[k], order[k + 1]
            events.setdefault((min(a, b), max(a, b), t), []).append(i)
    for (a, b, t), cols in events.items():
        shat = np.sign(ranks[b, t, :] - ranks[a, t, :])
        W = np.outer(shat, shat)
        mask = np.zeros((N, N), dtype=bool)
        mask[cols, :] = True
        mask[:, cols] = True
        C_fix += W * mask
    return C_fix.astype(np.float32)


def kernel(**inputs) -> np.ndarray:
    X = np.asarray(inputs["inputs"], dtype=np.float32)
    R = _ranks(X)
    nc = _get_nc()
    in_maps = [_prep_core_inputs(R, c) for c in range(NCORES)]
    res = run_bass_kernel_spmd(nc, in_maps, core_ids=list(range(NCORES)))
    G = np.zeros((N, N), dtype=np.float64)
    for c, r in enumerate(res.results):
        o = np.asarray(r["out"], dtype=np.float64)
        G += o[0:N, 0:N] + o[N:P, N:P]
        w = 0.5 if c == NCORES - 1 else 1.0
        G += (o[0:N, P:P + N] + o[N:P, P + N:2 * P]) * w
    S = _u_rowsums(R)
    total = 2016.0 * T
    C_half = 4.0 * G - 2.0 * (S[:, None] + S[None, :]) + total
    C_half = C_half.astype(np.float32) - _tie_correction(X, R)
    D = (1.0 - C_half / np.float32(PAIRS_HALF)) * (
        1.0 - np.eye(N, dtype=np.float32)
    )
    return np.ascontiguousarray(
        np.broadcast_to(D[None].astype(np.float32), (B, N, N))
    )


# revision 6
# speedup vs baseline: 1.0665x; 1.0665x over previous
"""Kendall distance kernel for Trainium2, SPMD over 8 NeuronCores.

Math: for X (B=64, T=256, N=64),
  C[i,j] = sum_{a,b,t} sign(X[b,t,i]-X[a,t,i]) * sign(X[b,t,j]-X[a,t,j])
  D = (1 - C/2016) * (1 - eye(N));  output = broadcast D to (B, N, N).

Device work: Gram of the sign tensor over all unordered batch pairs
(2016 pairs x 256 t rows), sharded across 8 cores by cyclic
batch-difference class: core c handles ring-offset classes
d in {4c+1 .. 4c+4}; class 32 (core 7, slot 3) covers its pairs twice,
so j=3 chunks go to a second PSUM accumulator the host halves.

Key trick vs the sub+sign baseline: inputs are per-column RANKS, which
are distinct integers, so sign(r_b - r_a) = 2*I(r_b > r_a) - 1 exactly.
The device computes ONE DVE `is_gt` op per chunk (u in {0,1} bf16) and
Grams u directly on PE.  The host expands
  sum s_i s_j = 4*sum u_i u_j - 2*(S_i + S_j) + count
with S[i] = row-sums of u computed on host from the ranks.  This halves
DVE work and removes all ACT usage (no Sign table load).

Slots are graduated (4,12,16,16,16 batch-blocks): the first chunks need
only a small DMA so the DVE pipeline starts ~5us earlier.  Slot-0
pieces ride the vector queue (DVE is idle until they land anyway);
later slots spread over scalar/gpsimd/tensor/sync queues.
"""

import numpy as np
import ml_dtypes

import concourse.bass as bass  # noqa: F401
import concourse.bacc as bacc
import concourse.tile as tile
from concourse import mybir
from concourse.bass_utils import run_bass_kernel_spmd

B, T, N = 64, 256, 64
P = 128
TH = T // P                   # 2
NCORES = 8
NSLOT = 4                     # classes per core (j = 0..3)
SLOT_NB = (16, 16, 16, 16)     # batch-blocks per slot (sum = 64)
NSLOTS = len(SLOT_NB)
SLOT_START = tuple(int(x) for x in np.cumsum((0,) + SLOT_NB[:-1]))
WEXT = NSLOT - 1              # extra blocks per x2 window (3)
BFD = TH * N                  # free elems per block (128)
PAIRS_HALF = 1008.0

_CACHE = {}


def _build_nc():
    nc = bacc.Bacc(
        "TRN2",
        target_bir_lowering=False,
        debug=False,
        num_devices=NCORES,
    )
    f32 = mybir.dt.float32
    bf16 = mybir.dt.bfloat16
    x1_dram = [
        nc.dram_tensor(f"x1s{s}", [P, nb * BFD], bf16, kind="ExternalInput")
        for s, nb in enumerate(SLOT_NB)
    ]
    x2_dram = [
        nc.dram_tensor(f"x2s{s}", [P, (nb + WEXT) * BFD], bf16, kind="ExternalInput")
        for s, nb in enumerate(SLOT_NB)
    ]
    out_dram = nc.dram_tensor("out", [P, 2 * P], f32, kind="ExternalOutput")

    with tile.TileContext(nc) as tc:
        with (
            tc.tile_pool(name="xpool", bufs=1) as xpool,
            tc.tile_pool(name="spool", bufs=6) as spool,
            tc.tile_pool(name="psum", bufs=2, space="PSUM") as psum,
            tc.tile_pool(name="opool", bufs=1) as opool,
        ):
            x1t = [
                xpool.tile([P, nb * BFD], bf16, tag=f"x1t{s}", name=f"x1t{s}")
                for s, nb in enumerate(SLOT_NB)
            ]
            x2t = [
                xpool.tile([P, (nb + WEXT) * BFD], bf16, tag=f"x2t{s}", name=f"x2t{s}")
                for s, nb in enumerate(SLOT_NB)
            ]
            # Queue plan: slots 0-2 alternate gpsimd/scalar (each queue's
            # pieces complete in order); slot 3 rides the sync queue, which
            # proved fast and otherwise idle, shortening the main queues.
            for s in (0, 1, 2):
                a, b = (nc.gpsimd, nc.scalar) if s % 2 == 0 else (nc.scalar, nc.gpsimd)
                a.dma_start(x2t[s][:, :], x2_dram[s][:, :])
                b.dma_start(x1t[s][:, :], x1_dram[s][:, :])
            nc.sync.dma_start(x2t[3][:, :], x2_dram[3][:, :])
            nc.sync.dma_start(x1t[3][:, :], x1_dram[3][:, :])

            c1_ps = psum.tile([P, P], f32, tag="c1")
            c2_ps = psum.tile([P, P], f32, tag="c2")
            n1 = (NSLOT - 1) * B // 2 * TH   # matmuls into c1 (192)
            n2 = B // 2 * TH                 # matmuls into c2 (64)
            k1 = k2 = 0
            for s, nb in enumerate(SLOT_NB):
                cfd = nb * BFD
                pmt = nb * TH // 2
                for j in range(NSLOT):
                    idx = s * NSLOT + j
                    sign = spool.tile([P, cfd], bf16, tag=f"sg{s}", name=f"sign{idx}")
                    nc.vector.tensor_tensor(
                        sign[:, :],
                        x2t[s][:, j * BFD:j * BFD + cfd],
                        x1t[s][:, :],
                        op=mybir.AluOpType.is_gt,
                    )
                    for m in range(pmt):
                        w_tile = sign[:, m * P:(m + 1) * P]
                        if j < NSLOT - 1:
                            st, sp = k1 == 0, k1 == n1 - 1
                            k1 += 1
                            acc = c1_ps
                        else:
                            st, sp = k2 == 0, k2 == n2 - 1
                            k2 += 1
                            acc = c2_ps
                        nc.tensor.matmul(
                            acc[:, :], w_tile, w_tile, start=st, stop=sp
                        )

            out_sb = opool.tile([P, 2 * P], f32)
            nc.vector.tensor_copy(out_sb[:, 0:P], c1_ps[:, :])
            nc.vector.tensor_copy(out_sb[:, P:2 * P], c2_ps[:, :])
            nc.sync.dma_start(out_dram[:, :], out_sb[:, :])

    nc.compile()
    return nc


def _get_nc():
    if "nc" not in _CACHE:
        _CACHE["nc"] = _build_nc()
    return _CACHE["nc"]


def _ranks(X):
    """Per-(t,i)-column batch ranks, 0..B-1, exact in bf16."""
    order = np.argsort(X, axis=0, kind="stable")
    ranks = np.empty_like(order)
    np.put_along_axis(
        ranks, order, np.arange(B, dtype=order.dtype)[:, None, None], axis=0
    )
    return ranks.astype(np.float32)


def _to_sbuf_layout(blocks):
    nb = blocks.shape[0]
    return np.ascontiguousarray(
        blocks.reshape(nb, TH, P, N)
        .transpose(2, 0, 1, 3)
        .reshape(P, nb * BFD)
        .astype(ml_dtypes.bfloat16)
    )


def _prep_core_inputs(R, c):
    r = np.roll(R, -(NSLOT * c + 1), axis=0)
    ext = np.concatenate([r, r[:WEXT]], axis=0)  # 67 blocks
    ins = {}
    for s, nb in enumerate(SLOT_NB):
        st = SLOT_START[s]
        ins[f"x1s{s}"] = _to_sbuf_layout(R[st:st + nb])
        ins[f"x2s{s}"] = _to_sbuf_layout(ext[st:st + nb + WEXT])
    return ins


def _u_rowsums(R):
    """Weighted row-sums S[i] = sum over device-covered ordered ring pairs
    (a, a+d), d=1..32 (d=32 halved), of I(R[a+d,t,i] > R[a,t,i])."""
    S = np.zeros(N, dtype=np.float64)
    for d in range(1, 33):
        w = 0.5 if d == 32 else 1.0
        S += w * (np.roll(R, -d, axis=0) > R).sum(axis=(0, 1))
    return S


def _tie_correction(X, ranks):
    """Exact fix for within-column value ties: the rank-sign kernel counts
    sign(rank diff)=+-1 where the true sign is 0."""
    C_fix = np.zeros((N, N), dtype=np.float64)
    Xs = np.sort(X, axis=0)
    t_idx, i_idx = np.nonzero((Xs[1:] == Xs[:-1]).any(axis=0))
    events = {}
    for t, i in zip(t_idx, i_idx):
        col = X[:, t, i]
        order = np.argsort(col, kind="stable")
        sc = col[order]
        for k in np.nonzero(sc[1:] == sc[:-1])[0]:
            a, b = order[k], order[k + 1]
            events.setdefault((min(a, b), max(a, b), t), []).append(i)
    for (a, b, t), cols in events.items():
        shat = np.sign(ranks[b, t, :] - ranks[a, t, :])
        W = np.outer(shat, shat)
        mask = np.zeros((N, N), dtype=bool)
        mask[cols, :] = True
        mask[:, cols] = True
        C_fix += W * mask
    return C_fix.astype(np.float32)


def kernel(**inputs) -> np.ndarray:
    X = np.asarray(inputs["inputs"], dtype=np.float32)
    R = _ranks(X)
    nc = _get_nc()
    in_maps = [_prep_core_inputs(R, c) for c in range(NCORES)]
    res = run_bass_kernel_spmd(nc, in_maps, core_ids=list(range(NCORES)))
    G = np.zeros((N, N), dtype=np.float64)
    for c, r in enumerate(res.results):
        o = np.asarray(r["out"], dtype=np.float64)
        G += o[0:N, 0:N] + o[N:P, N:P]
        w = 0.5 if c == NCORES - 1 else 1.0
        G += (o[0:N, P:P + N] + o[N:P, P + N:2 * P]) * w
    S = _u_rowsums(R)
    total = 2016.0 * T
    C_half = 4.0 * G - 2.0 * (S[:, None] + S[None, :]) + total
    C_half = C_half.astype(np.float32) - _tie_correction(X, R)
    D = (1.0 - C_half / np.float32(PAIRS_HALF)) * (
        1.0 - np.eye(N, dtype=np.float32)
    )
    return np.ascontiguousarray(
        np.broadcast_to(D[None].astype(np.float32), (B, N, N))
    )


# revision 7
# speedup vs baseline: 1.1363x; 1.0655x over previous
"""Kendall distance kernel for Trainium2, SPMD over 8 NeuronCores.

Math: for X (B=64, T=256, N=64),
  C[i,j] = sum_{a,b,t} sign(X[b,t,i]-X[a,t,i]) * sign(X[b,t,j]-X[a,t,j])
  D = (1 - C/2016) * (1 - eye(N));  output = broadcast D to (B, N, N).

Device work: Gram of the sign tensor over all unordered batch pairs
(2016 pairs x 256 t rows), sharded across 8 cores by cyclic
batch-difference class: core c handles ring-offset classes
d in {4c+1 .. 4c+4}; class 32 (core 7, slot 3) covers its pairs twice,
so j=3 chunks go to a second PSUM accumulator the host halves.

Key trick vs the sub+sign baseline: inputs are per-column RANKS, which
are distinct integers, so sign(r_b - r_a) = 2*I(r_b > r_a) - 1 exactly.
The device computes ONE DVE `is_gt` op per chunk (u in {0,1} bf16) and
Grams u directly on PE.  The host expands
  sum s_i s_j = 4*sum u_i u_j - 2*(S_i + S_j) + count
with S[i] = row-sums of u computed on host from the ranks.  This halves
DVE work and removes all ACT usage (no Sign table load).

Slots are graduated (4,12,16,16,16 batch-blocks): the first chunks need
only a small DMA so the DVE pipeline starts ~5us earlier.  Slot-0
pieces ride the vector queue (DVE is idle until they land anyway);
later slots spread over scalar/gpsimd/tensor/sync queues.
"""

import numpy as np
import ml_dtypes

import concourse.bass as bass  # noqa: F401
import concourse.bacc as bacc
import concourse.tile as tile
from concourse import mybir
from concourse.bass_utils import run_bass_kernel_spmd

B, T, N = 64, 256, 64
P = 128
TH = T // P                   # 2
NCORES = 8
NSLOT = 4                     # classes per core (j = 0..3)
SLOT_NB = (16, 16, 16, 16)     # batch-blocks per slot (sum = 64)
NSLOTS = len(SLOT_NB)
SLOT_START = tuple(int(x) for x in np.cumsum((0,) + SLOT_NB[:-1]))
WEXT = NSLOT - 1              # extra blocks per x2 window (3)
BFD = TH * N                  # free elems per block (128)
PAIRS_HALF = 1008.0

_CACHE = {}


def _build_nc():
    nc = bacc.Bacc(
        "TRN2",
        target_bir_lowering=False,
        debug=False,
        num_devices=NCORES,
    )
    f32 = mybir.dt.float32
    bf16 = mybir.dt.bfloat16
    x1_dram = [
        nc.dram_tensor(f"x1s{s}", [P, nb * BFD], bf16, kind="ExternalInput")
        for s, nb in enumerate(SLOT_NB)
    ]
    x2_dram = [
        nc.dram_tensor(f"x2s{s}", [P, (nb + WEXT) * BFD], bf16, kind="ExternalInput")
        for s, nb in enumerate(SLOT_NB)
    ]
    out_dram = nc.dram_tensor("out", [P, 2 * P], f32, kind="ExternalOutput")

    with tile.TileContext(nc) as tc:
        with (
            tc.tile_pool(name="xpool", bufs=1) as xpool,
            tc.tile_pool(name="spool", bufs=6) as spool,
            tc.tile_pool(name="psum", bufs=2, space="PSUM") as psum,
            tc.tile_pool(name="opool", bufs=1) as opool,
        ):
            x1t = [
                xpool.tile([P, nb * BFD], bf16, tag=f"x1t{s}", name=f"x1t{s}")
                for s, nb in enumerate(SLOT_NB)
            ]
            x2t = [
                xpool.tile([P, (nb + WEXT) * BFD], bf16, tag=f"x2t{s}", name=f"x2t{s}")
                for s, nb in enumerate(SLOT_NB)
            ]
            # Queue plan: alternate gpsimd/scalar so each chunk's two pieces
            # land together, in order (each queue sustains ~170-190GB/s).
            for s in range(NSLOTS):
                a, b = (nc.gpsimd, nc.scalar) if s % 2 == 0 else (nc.scalar, nc.gpsimd)
                a.dma_start(x2t[s][:, :], x2_dram[s][:, :])
                b.dma_start(x1t[s][:, :], x1_dram[s][:, :])

            c1_ps = psum.tile([P, P], f32, tag="c1")
            c2_ps = psum.tile([P, P], f32, tag="c2")
            n1 = (NSLOT - 1) * B // 2 * TH   # matmuls into c1 (192)
            n2 = B // 2 * TH                 # matmuls into c2 (64)
            k1 = k2 = 0
            for s, nb in enumerate(SLOT_NB):
                cfd = nb * BFD
                pmt = nb * TH // 2
                for j in range(NSLOT):
                    idx = s * NSLOT + j
                    sign = spool.tile([P, cfd], bf16, tag=f"sg{s}", name=f"sign{idx}")
                    nc.vector.tensor_tensor(
                        sign[:, :],
                        x2t[s][:, j * BFD:j * BFD + cfd],
                        x1t[s][:, :],
                        op=mybir.AluOpType.is_gt,
                    )
                    for m in range(pmt):
                        w_tile = sign[:, m * P:(m + 1) * P]
                        if j < NSLOT - 1:
                            st, sp = k1 == 0, k1 == n1 - 1
                            k1 += 1
                            acc = c1_ps
                        else:
                            st, sp = k2 == 0, k2 == n2 - 1
                            k2 += 1
                            acc = c2_ps
                        nc.tensor.matmul(
                            acc[:, :], w_tile, w_tile, start=st, stop=sp
                        )

            out_sb = opool.tile([P, 2 * P], f32)
            nc.vector.tensor_copy(out_sb[:, 0:P], c1_ps[:, :])
            nc.vector.tensor_copy(out_sb[:, P:2 * P], c2_ps[:, :])
            nc.sync.dma_start(out_dram[:, :], out_sb[:, :])

    nc.compile()
    return nc


def _get_nc():
    if "nc" not in _CACHE:
        _CACHE["nc"] = _build_nc()
    return _CACHE["nc"]


def _ranks(X):
    """Per-(t,i)-column batch ranks, 0..B-1, exact in bf16."""
    order = np.argsort(X, axis=0, kind="stable")
    ranks = np.empty_like(order)
    np.put_along_axis(
        ranks, order, np.arange(B, dtype=order.dtype)[:, None, None], axis=0
    )
    return ranks.astype(np.float32)


def _to_sbuf_layout(blocks):
    nb = blocks.shape[0]
    return np.ascontiguousarray(
        blocks.reshape(nb, TH, P, N)
        .transpose(2, 0, 1, 3)
        .reshape(P, nb * BFD)
        .astype(ml_dtypes.bfloat16)
    )


def _prep_core_inputs(R, c):
    r = np.roll(R, -(NSLOT * c + 1), axis=0)
    ext = np.concatenate([r, r[:WEXT]], axis=0)  # 67 blocks
    ins = {}
    for s, nb in enumerate(SLOT_NB):
        st = SLOT_START[s]
        ins[f"x1s{s}"] = _to_sbuf_layout(R[st:st + nb])
        ins[f"x2s{s}"] = _to_sbuf_layout(ext[st:st + nb + WEXT])
    return ins


def _u_rowsums(R):
    """Weighted row-sums S[i] = sum over device-covered ordered ring pairs
    (a, a+d), d=1..32 (d=32 halved), of I(R[a+d,t,i] > R[a,t,i])."""
    S = np.zeros(N, dtype=np.float64)
    for d in range(1, 33):
        w = 0.5 if d == 32 else 1.0
        S += w * (np.roll(R, -d, axis=0) > R).sum(axis=(0, 1))
    return S


def _tie_correction(X, ranks):
    """Exact fix for within-column value ties: the rank-sign kernel counts
    sign(rank diff)=+-1 where the true sign is 0."""
    C_fix = np.zeros((N, N), dtype=np.float64)
    Xs = np.sort(X, axis=0)
    t_idx, i_idx = np.nonzero((Xs[1:] == Xs[:-1]).any(axis=0))
    events = {}
    for t, i in zip(t_idx, i_idx):
        col = X[:, t, i]
        order = np.argsort(col, kind="stable")
        sc = col[order]
        for k in np.nonzero(sc[1:] == sc[:-1])[0]:
            a, b = order[k], order[k + 1]
            events.setdefault((min(a, b), max(a, b), t), []).append(i)
    for (a, b, t), cols in events.items():
        shat = np.sign(ranks[b, t, :] - ranks[a, t, :])
        W = np.outer(shat, shat)
        mask = np.zeros((N, N), dtype=bool)
        mask[cols, :] = True
        mask[:, cols] = True
        C_fix += W * mask
    return C_fix.astype(np.float32)


def kernel(**inputs) -> np.ndarray:
    X = np.asarray(inputs["inputs"], dtype=np.float32)
    R = _ranks(X)
    nc = _get_nc()
    in_maps = [_prep_core_inputs(R, c) for c in range(NCORES)]
    res = run_bass_kernel_spmd(nc, in_maps, core_ids=list(range(NCORES)))
    G = np.zeros((N, N), dtype=np.float64)
    for c, r in enumerate(res.results):
        o = np.asarray(r["out"], dtype=np.float64)
        G += o[0:N, 0:N] + o[N:P, N:P]
        w = 0.5 if c == NCORES - 1 else 1.0
        G += (o[0:N, P:P + N] + o[N:P, P + N:2 * P]) * w
    S = _u_rowsums(R)
    total = 2016.0 * T
    C_half = 4.0 * G - 2.0 * (S[:, None] + S[None, :]) + total
    C_half = C_half.astype(np.float32) - _tie_correction(X, R)
    D = (1.0 - C_half / np.float32(PAIRS_HALF)) * (
        1.0 - np.eye(N, dtype=np.float32)
    )
    return np.ascontiguousarray(
        np.broadcast_to(D[None].astype(np.float32), (B, N, N))
    )
